# revision 1
# baseline (speedup 1.0000x reference)
"""CoNystromAttention Trainium2 kernel.

Shard: 8 cores = 4 batches x 2 head-groups (8 heads each). Per core:
one batch b, 8 heads organized as 4 "pairs" (2 heads = 128 partitions).

Math (reference, with Q=K=V=QKV):
  QKV = X[b].T @ Wq[h].T + bq[h]                       [n=4096, d=64]
  Qt  = window-mean(QKV, 64)                           [m=64, d]
  S   = exp(QKV @ Qt.T / 8)     (Beta; Delta = S.T)    [n, m]
  G   = exp(Qt @ Qt.T / 8)                             [m, m]
  GD  = G / rowsum(G);  V6 = newton_schulz(GD, 6)      (pinv)
  out = diag(1/r) S V6 diag(1/c) S.T QKV,  r=rowsum(S), c=colsum(S)

All big matmuls in float32r (tf32-like, full PE rate at N>=256).
"""

import numpy as np

P = 128
N_TOK = 4096
EMBED = 1024
NPAIR = 4            # head-pairs per core (8 heads)
ECH = EMBED // P     # 8 contraction chunks
XCH = 256            # projection chunk (tokens)
NCHP = N_TOK // XCH  # 16 projection chunks
NCH8 = N_TOK // 512  # 8 ST chunks of 512
TCH = N_TOK // P     # 32 token chunks of 128
NS_ITERS = 6

_CACHE = {}


def _build(global_scale=True):
    import concourse.mybir as mybir
    from concourse import bacc, bass_isa
    from concourse.tile import TileContext
    from concourse.masks import make_identity

    f32 = mybir.dt.float32
    f32r = mybir.dt.float32r
    ALU = mybir.AluOpType
    ACTF = mybir.ActivationFunctionType
    AX = mybir.AxisListType

    nc = bacc.Bacc("TRN2", target_bir_lowering=False, debug=False)
    X = nc.dram_tensor("X", [EMBED, N_TOK], f32, kind="ExternalInput")
    WqT = nc.dram_tensor("WqT", [EMBED, 512], f32, kind="ExternalInput")
    bias = nc.dram_tensor("bias", [512], f32, kind="ExternalInput")
    out_d = nc.dram_tensor("out", [N_TOK, 512], f32, kind="ExternalOutput")
    if global_scale:
        cc_in = nc.dram_tensor("cc_in", [1, 1], f32)
        cc_out = nc.dram_tensor("cc_out", [1, 1], f32, addr_space="Shared")

    with TileContext(nc) as tc, (
        tc.tile_pool(name="big", bufs=1)
    ) as big, tc.tile_pool(name="persist", bufs=1) as pers, tc.tile_pool(
        name="nsv", bufs=1
    ) as nsp:
        # ---------------- persistent tiles ----------------
        ident32 = pers.tile([P, P], f32, tag="ident32")
        make_identity(nc, ident32[:])
        identr = pers.tile([P, P], f32r, tag="identr")
        nc.vector.tensor_copy(identr[:], ident32[:])
        i7 = pers.tile([P, 256], f32, tag="i7")
        i15 = pers.tile([P, 256], f32, tag="i15")
        i13 = pers.tile([P, 256], f32, tag="i13")
        for t, v in ((i7, 7.0), (i15, 15.0), (i13, 13.0)):
            nc.vector.memset(t[:], 0.0)
            nc.vector.tensor_scalar_mul(t[:, :P], ident32[:], v)
        bias_t = pers.tile([P, NPAIR], f32, tag="bias")
        nc.sync.dma_start(bias_t[:], bias.rearrange("(f p) -> p f", p=P))
        zsrc = pers.tile([P, P], f32, tag="zsrc")
        nc.vector.memset(zsrc[:], 0.0)
        qsum = [pers.tile([P, 64], f32, tag=f"qsum{p}", name=f"qsum{p}") for p in range(NPAIR)]
        qkvt = big.tile([P, NPAIR, N_TOK], f32r, tag="qkvt")
        st = big.tile([P, NPAIR, N_TOK], f32r, tag="st")

        # ---------------- phase 1: projection ----------------
        with (
            tc.tile_pool(name="wq", bufs=1) as wqp,
            tc.tile_pool(name="x", bufs=2) as xpool,
            tc.tile_pool(name="x2", bufs=3) as xpool2,
            tc.tile_pool(name="pp", bufs=8, space="PSUM") as pp,
        ):
            wqtr = wqp.tile([P, ECH, 512], f32r, tag="wqtr")
            for half in range(2):
                for ch in range(2):
                    stg = xpool.tile([P, ECH // 2, XCH], f32, tag="xt")
                    nc.sync.dma_start(
                        stg[:],
                        WqT.rearrange("(eo p) hd -> p eo hd", p=P)[
                            :, half * 4:(half + 1) * 4, ch * 256:(ch + 1) * 256
                        ],
                    )
                    nc.vector.tensor_copy(
                        wqtr[:, half * 4:(half + 1) * 4, ch * 256:(ch + 1) * 256],
                        stg[:],
                    )

            xre = X.rearrange("(eo p) n -> p eo n", p=P)
            for c in range(NCHP):
                xrs = []
                for half in range(2):
                    xt = xpool.tile([P, ECH // 2, XCH], f32, tag="xt")
                    nc.sync.dma_start(
                        xt[:],
                        xre[:, half * 4:(half + 1) * 4, c * XCH:(c + 1) * XCH],
                    )
                    xr = xpool2.tile([P, ECH // 2, XCH], f32r, tag="xr")
                    nc.scalar.copy(xr[:], xt[:])
                    xrs.append(xr)
                for p in range(NPAIR):
                    ps = pp.tile([P, XCH], f32, tag="proj")
                    for e in range(ECH):
                        nc.tensor.matmul(
                            ps[:],
                            wqtr[:, e, p * P:(p + 1) * P],
                            xrs[e // 4][:, e % 4, :],
                            start=(e == 0),
                            stop=(e == ECH - 1),
                        )
                    nc.vector.tensor_scalar_add(
                        qkvt[:, p, c * XCH:(c + 1) * XCH], ps[:], bias_t[:, p:p + 1]
                    )
                    # landmark partial sums (pre-rounding, no bias): 4 windows/chunk
                    nc.vector.reduce_sum(
                        qsum[p][:, c * 4:(c + 1) * 4],
                        ps[:].rearrange("p (w t) -> p w t", t=64),
                        axis=AX.X,
                    )

        # ---------------- phase 2 ----------------
        with (
            tc.tile_pool(name="wk", bufs=4) as wk,
            tc.tile_pool(name="sn", bufs=4) as snp,
            tc.tile_pool(name="nsps", bufs=3, space="PSUM") as nsps,
            tc.tile_pool(name="trps", bufs=3, space="PSUM") as trps,
            tc.tile_pool(name="mps", bufs=1, space="PSUM") as mps,
        ):
            # landmarks (Qt~ = qsum/64 + bias), block-diagonal per pair
            blkq = []
            for p in range(NPAIR):
                bq_t = pers.tile([P, P], f32r, tag=f"blkq{p}")
                nc.vector.tensor_copy(bq_t[0:64, 64:128], zsrc[0:64, 0:64])
                nc.vector.tensor_copy(bq_t[64:128, 0:64], zsrc[0:64, 0:64])
                nc.vector.tensor_scalar(
                    bq_t[0:64, 0:64], qsum[p][0:64, :], 1.0 / 64,
                    bias_t[0:64, p:p + 1], ALU.mult, ALU.add,
                )
                nc.vector.tensor_scalar(
                    bq_t[64:128, 64:128], qsum[p][64:128, :], 1.0 / 64,
                    bias_t[64:128, p:p + 1], ALU.mult, ALU.add,
                )
                blkq.append(bq_t)

            # Gamma -> GD -> Newton-Schulz init
            if global_scale:
                gstage = pers.tile([1, 2 * NPAIR], f32, tag="gstage")
                ones_row = pers.tile([1, P], f32, tag="ones_row")
                nc.vector.memset(ones_row[:], 1.0)
            vstate = []
            for p in range(NPAIR):
                psg = nsps.tile([P, 256], f32, tag="nsb")
                nc.tensor.matmul(psg[:, :P], blkq[p][:], blkq[p][:], start=True, stop=True)
                g = wk.tile([P, P], f32, tag="g")
                nc.scalar.activation(g[:], psg[:, :P], ACTF.Exp, scale=0.125)
                nc.vector.memset(g[0:64, 64:128], 0.0)
                nc.vector.memset(g[64:128, 0:64], 0.0)
                gs = wk.tile([P, 1], f32, tag="gs")
                nc.vector.reduce_sum(gs[:], g[:], axis=AX.X)
                gri = wk.tile([P, 1], f32, tag="gri")
                nc.vector.reciprocal(gri[:], gs[:])
                gd = nsp.tile([P, P], f32, tag=f"gd{p}", name=f"gd{p}")
                nc.vector.tensor_scalar_mul(gd[:], g[:], gri[:])

                pskt = nsps.tile([P, 256], f32, tag="nsb")
                nc.tensor.matmul(pskt[:, :P], gd[:], ident32[:], is_transpose=True)
                ktpad = nsp.tile([P, 256], f32r, tag=f"kt{p}")
                nc.vector.tensor_copy(ktpad[:, P:], zsrc[:])
                csum = wk.tile([P, 1], f32, tag="csum")
                nc.vector.tensor_scalar(
                    ktpad[:, :P], pskt[:, :P], 1.0, None, ALU.mult, ALU.add, accum_out=csum[:]
                )
                # partition_all_reduce only works at base partition 0:
                # separate the two heads into columns, pad with -1e30
                csep = wk.tile([P, 2], f32, tag="csep")
                nc.vector.memset(csep[:], -1e30)
                nc.vector.tensor_copy(csep[0:64, 0:1], csum[0:64, :])
                nc.vector.tensor_copy(csep[64:128, 1:2], csum[64:128, :])
                cmax = wk.tile([P, 2], f32, tag="cmax")
                nc.gpsimd.partition_all_reduce(
                    cmax[:], csep[:], P, bass_isa.ReduceOp.max
                )
                if global_scale:
                    nc.vector.tensor_copy(gstage[0:1, 2 * p:2 * p + 2], cmax[0:1, 0:2])
                    sv = None
                else:
                    sv = wk.tile([P, 1], f32, tag="sv")
                    nc.vector.reciprocal(sv[0:64, :], cmax[0:64, 0:1])
                    nc.vector.reciprocal(sv[64:128, :], cmax[64:128, 1:2])
                vstate.append([ktpad, sv, gd])

            if global_scale:
                gmax = pers.tile([1, 1], f32, tag="gmax")
                nc.vector.reduce_max(gmax[:], gstage[:], axis=AX.X)
                nc.sync.dma_start(cc_in.ap(), gmax[:])
                nc.gpsimd.collective_compute(
                    "AllReduce", ALU.max, [list(range(8))],
                    ins=[cc_in.ap().opt()], outs=[cc_out.ap().opt()],
                )
                gback = pers.tile([1, 1], f32, tag="gback")
                nc.sync.dma_start(gback[:], cc_out.ap())
                psb = nsps.tile([P, 256], f32, tag="nsb")
                nc.tensor.matmul(psb[:, 0:1], ones_row[:], gback[:], start=True, stop=True)
                sv_g = pers.tile([P, 1], f32, tag="sv_g")
                nc.vector.reciprocal(sv_g[:], psb[:, 0:1])

            for p in range(NPAIR):
                ktpad, sv, gd = vstate[p]
                if global_scale:
                    sv = sv_g
                v0 = nsp.tile([P, 256], f32r, tag=f"v{p}", name=f"v0_{p}")
                nc.vector.tensor_copy(v0[:, P:], zsrc[:])
                nc.vector.tensor_scalar_mul(v0[:, :P], ktpad[:, :P], sv[:])
                # V0^T = s*K directly (s constant within each head block)
                vt0 = nsp.tile([P, 256], f32r, tag=f"vt{p}", name=f"vt0_{p}")
                nc.vector.tensor_copy(vt0[:, P:], zsrc[:])
                nc.vector.tensor_scalar_mul(vt0[:, :P], gd[:], sv[:])
                vstate[p] = [ktpad, v0, vt0]

            # Newton-Schulz iterations (fp32r, right halves stay zero).
            # it-outer so the four independent pair-chains pipeline.
            vcur = [list(vstate[p]) for p in range(NPAIR)]
            for it in range(NS_ITERS):
                for p in range(NPAIR):
                    pool_a, tag_a = nsps, "nsb"
                    pool_b, tag_b = nsps, "nsb"
                    ktpad, v, vt = vcur[p]
                    pskv = pool_a.tile([P, 256], f32, tag=tag_a, name=f"pskv{p}_{it}")
                    nc.tensor.matmul(pskv[:], ktpad[:, :P], v[:], start=True, stop=True)
                    pskvt = pool_b.tile([P, 256], f32, tag=tag_b, name=f"pskvt{p}_{it}")
                    nc.tensor.matmul(pskvt[:], v[:, :P], ktpad[:], start=True, stop=True)
                    kvt = nsp.tile([P, 256], f32r, tag=f"kvt{p}", name=f"kvt{p}_{it}")
                    nc.vector.tensor_copy(kvt[:], pskvt[:])
                    a1 = nsp.tile([P, 256], f32r, tag=f"a1{p}", name=f"a1{p}_{it}")
                    nc.vector.tensor_tensor(a1[:], i7[:], pskv[:], ALU.subtract)
                    psa2 = pool_a.tile([P, 256], f32, tag=tag_a, name=f"psa2{p}_{it}")
                    nc.tensor.matmul(psa2[:], kvt[:, :P], a1[:], start=True, stop=True)
                    a3 = nsp.tile([P, 256], f32r, tag=f"a3{p}", name=f"a3{p}_{it}")
                    nc.vector.tensor_tensor(a3[:], i15[:], psa2[:], ALU.subtract)
                    psa4 = pool_b.tile([P, 256], f32, tag=tag_b, name=f"psa4{p}_{it}")
                    nc.tensor.matmul(psa4[:], kvt[:, :P], a3[:], start=True, stop=True)
                    a5 = nsp.tile([P, 256], f32r, tag=f"a5{p}", name=f"a5{p}_{it}")
                    nc.vector.tensor_tensor(a5[:], i13[:], psa4[:], ALU.subtract)
                    if it < NS_ITERS - 1:
                        psv = pool_a.tile([P, 256], f32, tag=tag_a, name=f"psv{p}_{it}")
                        nc.tensor.matmul(psv[:], vt[:, :P], a5[:], start=True, stop=True)
                        vn = nsp.tile([P, 256], f32r, tag=f"v{p}", name=f"vn{p}_{it}")
                        nc.vector.tensor_scalar_mul(vn[:], psv[:], 0.25)
                    else:
                        # v unused after the last iteration (W needs only vt)
                        vn = vcur[p][1]
                    psvt2 = pool_b.tile([P, 256], f32, tag=tag_b, name=f"psvt2{p}_{it}")
                    nc.tensor.matmul(psvt2[:], a5[:, :P], vt[:], start=True, stop=True)
                    vtn = nsp.tile([P, 256], f32r, tag=f"vt{p}", name=f"vtn{p}_{it}")
                    nc.vector.tensor_scalar_mul(vtn[:], psvt2[:], 0.25)
                    vcur[p] = [ktpad, vn, vtn]
            for p in range(NPAIR):
                vstate[p] = list(vcur[p])

            # ST = exp(blkQ^T @ QKVT / 8); c partials via accum_out
            cparts = []
            for p in range(NPAIR):
                cp = pers.tile([P, NCH8], f32, tag=f"cpart{p}")
                cparts.append(cp)
                for c in range(NCH8):
                    psst = trps.tile([P, 512], f32, tag="trp")
                    nc.tensor.matmul(
                        psst[:], blkq[p][:], qkvt[:, p, c * 512:(c + 1) * 512],
                        start=True, stop=True,
                    )
                    nc.scalar.activation(
                        st[:, p, c * 512:(c + 1) * 512], psst[:], ACTF.Exp,
                        scale=0.125, accum_out=cp[:, c:c + 1],
                    )

            # token-chunk loop: transposes + S-normal + M accumulation
            rv = pers.tile([P, 2 * NPAIR, TCH], f32, tag="rv")
            mbank = [mps.tile([P, 512], f32, tag=f"mb{q}", name=f"mb{q}") for q in range(2)]
            for c in range(TCH):
                tsl = slice(c * P, (c + 1) * P)
                psq = trps.tile([P, 512], f32r, tag="trp")
                for p in range(NPAIR):
                    nc.tensor.matmul(
                        psq[:, p * P:(p + 1) * P], qkvt[:, p, tsl], identr[:],
                        is_transpose=True, start=(p == 0), stop=(p == NPAIR - 1),
                        skip_group_check=True,
                    )
                qnb = snp.tile([P, 512], f32r, tag="qnb", name=f"qnb_{c}")
                nc.scalar.copy(qnb[:], psq[:])
                qn = [qnb[:, 0:256], qnb[:, 256:512]]
                pss = trps.tile([P, 512], f32r, tag="trp")
                for p in range(NPAIR):
                    nc.tensor.matmul(
                        pss[:, p * P:(p + 1) * P], st[:, p, tsl], identr[:],
                        is_transpose=True, start=(p == 0), stop=(p == NPAIR - 1),
                        skip_group_check=True,
                    )
                sn = [snp.tile([P, P], f32r, tag=f"sn{p}", name=f"sn{p}_{c}") for p in range(NPAIR)]
                for p in range(NPAIR):
                    nc.vector.tensor_scalar(
                        sn[p][:, 0:64], pss[:, p * P:p * P + 64], 1.0, None,
                        ALU.mult, ALU.add, accum_out=rv[:, 2 * p, c:c + 1],
                    )
                    nc.vector.tensor_scalar(
                        sn[p][:, 64:128], pss[:, p * P + 64:(p + 1) * P], 1.0, None,
                        ALU.mult, ALU.add, accum_out=rv[:, 2 * p + 1, c:c + 1],
                    )
                for q in range(2):
                    for j in range(2):
                        p = 2 * q + j
                        nc.tensor.matmul(
                            mbank[q][:, j * 256:(j + 1) * 256], sn[p][:], qn[q],
                            start=(c == 0 and j == 0),
                            stop=(c == TCH - 1 and j == 1),
                            skip_group_check=True,
                        )

            nc.vector.reciprocal(rv[:], rv[:])

            # W = V6 @ (diag(1/c) M)
            wpads = []
            for p in range(NPAIR):
                q, j = divmod(p, 2)
                cs = wk.tile([P, 1], f32, tag="cs")
                nc.vector.reduce_sum(cs[:], cparts[p][:], axis=AX.X)
                cinv = wk.tile([P, 1], f32, tag="cinv")
                nc.vector.reciprocal(cinv[:], cs[:])
                dvp = wk.tile([P, 256], f32r, tag="dvp")
                nc.vector.tensor_copy(dvp[:, P:], zsrc[:])
                nc.vector.tensor_scalar_mul(
                    dvp[:, :P], mbank[q][:, j * 384:j * 384 + P], cinv[:]
                )
                # zero cross-head blocks (garbage from the paired-rhs M matmul)
                nc.vector.tensor_copy(dvp[0:64, 64:128], zsrc[0:64, 0:64])
                nc.vector.tensor_copy(dvp[64:128, 0:64], zsrc[0:64, 0:64])
                psw = nsps.tile([P, 256], f32, tag="nsb")
                _, v6, vt6 = vstate[p]
                nc.tensor.matmul(psw[:], vt6[:, :P], dvp[:], start=True, stop=True)
                wpad = pers.tile([P, 256], f32r, tag=f"wpad{p}")
                nc.vector.tensor_copy(wpad[:], psw[:])
                wpads.append(wpad)

            # final: out = diag(1/r) S W  (2 pairs packed per psum bank)
            for c in range(TCH):
                tsl = slice(c * P, (c + 1) * P)
                for q in range(2):
                    pso = trps.tile([P, 512], f32, tag="trp", name=f"pso{q}_{c}")
                    for j in range(2):
                        p = 2 * q + j
                        nc.tensor.matmul(
                            pso[:, j * 256:j * 256 + 256], st[:, p, tsl], wpads[p][:],
                            start=(j == 0), stop=(j == 1), skip_group_check=True,
                        )
                    ot = wk.tile([P, 256], f32, tag="ot", name=f"ot{q}_{c}")
                    nc.vector.tensor_tensor(
                        ot[:].rearrange("p (b h d) -> p b h d", h=2, d=64),
                        pso[:].rearrange("p (b n) -> p b n", n=256)[:, :, 0:128]
                            .rearrange("p b (h d) -> p b h d", d=64),
                        rv[:, 4 * q:4 * q + 4, c:c + 1]
                            .rearrange("p (b h) one -> p b h one", h=2)
                            .to_broadcast([P, 2, 2, 64]),
                        ALU.mult,
                    )
                    nc.sync.dma_start(out_d[tsl, q * 256:(q + 1) * 256], ot[:])

    nc.compile()
    return nc


def _get_nc():
    if "nc" not in _CACHE:
        _CACHE["nc"] = _build()
    return _CACHE["nc"]


def kernel(X, Wq, bq):
    from concourse.bass_utils import run_bass_kernel_spmd

    nc = _get_nc()
    B, E, n = X.shape
    H = Wq.shape[0]
    in_maps = []
    for core in range(8):
        b = core // 2
        h0 = 8 * (core % 2)
        wq_c = Wq[h0:h0 + 8]                      # [8, 64, 1024]
        wqt_c = np.ascontiguousarray(wq_c.transpose(2, 0, 1).reshape(E, 512))
        bias_c = np.ascontiguousarray(bq[h0:h0 + 8].reshape(512))
        in_maps.append({
            "X": np.ascontiguousarray(X[b]),
            "WqT": wqt_c,
            "bias": bias_c,
        })
    res = run_bass_kernel_spmd(nc, in_maps, core_ids=list(range(8)))
    out = np.empty((B, H, n, 64), dtype=np.float32)
    for core in range(8):
        b = core // 2
        h0 = 8 * (core % 2)
        oc = res.results[core]["out"].reshape(n, 8, 64)
        out[b, h0:h0 + 8] = oc.transpose(1, 0, 2)
    return out



# revision 12
# speedup vs baseline: 1.3589x; 1.3589x over previous
"""CoNystromAttention Trainium2 kernel (v2).

Shard: 8 cores = 4 batches x 2 head-groups (8 heads each). Per core:
one batch b, 8 heads organized as 4 "pairs" (2 heads = 128 partitions).

Math (reference, with Q=K=V=QKV):
  QKV = X[b].T @ Wq[h].T + bq[h]                       [n=4096, d=64]
  Qt  = window-mean(QKV, 64)                           [m=64, d]
  S   = exp(QKV @ Qt.T / 8)     (Beta; Delta = S.T)    [n, m]
  G   = exp(Qt @ Qt.T / 8)
  GD  = G / rowsum(G);  V6 = newton_schulz(GD, 6)      (pinv)
  out = diag(1/r) S V6 diag(1/c) S.T QKV,  r=rowsum(S), c=colsum(S)

v2 design notes:
- NS init scale: rowsum(GD) == 1, so scale = 1/max(colsum) PER HEAD
  (verified: 1.3e-3 vs the reference's global max -> no collective).
- Streams (qkvt/st/qn + NS internals) in bf16: full-rate N=128 matmuls,
  1 cyc/row transposes, 2x DVE copies. Verified combined error ~7e-3.
- r (Beta rowsums) come out of the final matmul for free via ones
  columns appended to W (cols 128/129 of the 130-wide rhs).
- Engine balance: Act = proj-write+bias, ST exp(+c accum), NS copies;
  Pool = landmark window sums, half the final scaling; DVE = the rest.
"""

import numpy as np

P = 128
N_TOK = 4096
EMBED = 1024
NPAIR = 4            # head-pairs per core (8 heads)
ECH = EMBED // P     # 8 contraction chunks
XCH = 512            # projection chunk (tokens)
NCHP = N_TOK // XCH  # 8 projection chunks
TCH = N_TOK // P     # 32 token chunks of 128
NS_ITERS = 6

_CACHE = {}


def _build(global_scale=False, debug=False):
    del global_scale  # kept for test.py compat; no collective in v2
    import concourse.mybir as mybir
    from concourse import bacc
    from concourse.tile import TileContext
    from concourse.masks import make_identity

    f32 = mybir.dt.float32
    f32r = mybir.dt.float32r
    bf16 = mybir.dt.bfloat16
    ALU = mybir.AluOpType
    ACTF = mybir.ActivationFunctionType
    AX = mybir.AxisListType

    nc = bacc.Bacc("TRN2", target_bir_lowering=False, debug=False)
    X = nc.dram_tensor("X", [EMBED, N_TOK], f32r, kind="ExternalInput")
    WqT = nc.dram_tensor("WqT", [EMBED, 512], f32r, kind="ExternalInput")
    bias = nc.dram_tensor("bias", [512], f32, kind="ExternalInput")
    out_d = nc.dram_tensor("out", [N_TOK, 512], f32, kind="ExternalOutput")
    if debug:
        dbg = nc.dram_tensor("dbg", [P, 8192], f32, kind="ExternalOutput")

    with TileContext(nc) as tc, (
        tc.tile_pool(name="pers", bufs=1)
    ) as pers, tc.tile_pool(name="big", bufs=1) as big:
        # ---------------- persistent tiles ----------------
        ident32 = pers.tile([P, P], f32, tag="ident32")
        make_identity(nc, ident32[:])
        identb = pers.tile([P, P], bf16, tag="identb")
        nc.vector.tensor_copy(identb[:], ident32[:])
        # packed a*I | a*I constants for 2-pair NS elementwise
        i7 = pers.tile([P, 256], f32, tag="i7")
        i15 = pers.tile([P, 256], f32, tag="i15")
        i13 = pers.tile([P, 256], f32, tag="i13")
        for t, v in ((i7, 7.0), (i15, 15.0), (i13, 13.0)):
            nc.vector.tensor_scalar_mul(t[:, 0:P], ident32[:], v)
            nc.vector.tensor_scalar_mul(t[:, P:256], ident32[:], v)
        ones_col = pers.tile([P, 1], f32r, tag="ones_col")
        nc.vector.memset(ones_col[:], 1.0)
        ones_row = pers.tile([1, P], f32r, tag="ones_row")
        nc.vector.memset(ones_row[:], 1.0)
        bias_t = pers.tile([P, NPAIR], f32, tag="bias")
        nc.sync.dma_start(bias_t[:], bias.rearrange("(f p) -> p f", p=P))
        qsum = [pers.tile([P, 64], f32, tag=f"qsum{p}", name=f"qsum{p}") for p in range(NPAIR)]
        cp = [pers.tile([P, NCHP], f32, tag=f"cp{p}", name=f"cp{p}") for p in range(NPAIR)]
        qkvt = big.tile([P, NPAIR, N_TOK], bf16, tag="qkvt")
        st = big.tile([P, NPAIR, N_TOK], bf16, tag="st")
        qn = big.tile([P, TCH, 512], bf16, tag="qn")
        wqtr = pers.tile([P, ECH, 512], f32r, tag="wqtr")
        nc.sync.dma_start(wqtr[:], WqT.rearrange("(eo p) hd -> p eo hd", p=P))

        # ---------------- phase B: projection ----------------
        with (
            tc.tile_pool(name="x", bufs=2) as xpool,
            tc.tile_pool(name="pp", bufs=4, space="PSUM") as pp,
            tc.tile_pool(name="tq", bufs=2, space="PSUM") as tq,
        ):
            xre = X.rearrange("(eo p) n -> p eo n", p=P)
            for c in range(NCHP):
                xt = xpool.tile([P, ECH, XCH], f32r, tag="xt", name=f"xt{c}")
                nc.sync.dma_start(xt[:], xre[:, :, c * XCH:(c + 1) * XCH])
                csl = slice(c * XCH, (c + 1) * XCH)
                for p in range(NPAIR):
                    ps = pp.tile([P, XCH], f32, tag="proj", name=f"proj{c}_{p}")
                    for e in range(ECH):
                        nc.tensor.matmul(
                            ps[:],
                            wqtr[:, e, p * P:(p + 1) * P],
                            xt[:, e, :],
                            start=(e == 0),
                            stop=(e == ECH - 1),
                        )
                    # QKV^T (bf16) = psum + bias (per-partition)
                    nc.scalar.activation(
                        qkvt[:, p, csl], ps[:], ACTF.Identity,
                        bias=bias_t[:, p:p + 1],
                    )
                    # landmark window sums (64-token windows, pre-bias f32)
                    nc.vector.reduce_sum(
                        qsum[p][:, c * 8:(c + 1) * 8],
                        ps[:].rearrange("p (w t) -> p w t", t=64),
                        axis=AX.X,
                    )
                # qn: transpose QKV^T chunks -> [tok, hd]
                for sc in range(4):
                    t0 = c * 4 + sc
                    tsl = slice(t0 * P, (t0 + 1) * P)
                    psQ = tq.tile([P, 512], bf16, tag="psQ", name=f"psQ{t0}")
                    for p in range(NPAIR):
                        nc.tensor.matmul(
                            psQ[:, p * P:(p + 1) * P], qkvt[:, p, tsl], identb[:],
                            is_transpose=True, start=(p == 0), stop=(p == NPAIR - 1),
                            skip_group_check=True,
                        )
                    nc.vector.tensor_copy(qn[:, t0, :], psQ[:])

        # ---------------- phases C-F ----------------
        with (
            tc.tile_pool(name="wk", bufs=4) as wk,
            tc.tile_pool(name="nsv", bufs=1) as nsp,
            tc.tile_pool(name="sn", bufs=3) as snp,
            tc.tile_pool(name="ot", bufs=3) as otp,
            tc.tile_pool(name="mps", bufs=1, space="PSUM") as mps,
        ):
            cctx = tc.tile_pool(name="cps", bufs=1, space="PSUM")
            cps = cctx.__enter__()
            # ---- C: landmarks, Gamma, GD, NS init (per-head scale) ----
            blkq = []
            blkqb = []
            for p in range(NPAIR):
                bq_t = pers.tile([P, P], f32r, tag=f"blkq{p}", name=f"blkq{p}")
                nc.vector.memset(bq_t[:], 0.0)
                nc.vector.tensor_scalar(
                    bq_t[0:64, 0:64], qsum[p][0:64, :], 1.0 / 64,
                    bias_t[0:64, p:p + 1], ALU.mult, ALU.add,
                )
                nc.vector.tensor_scalar(
                    bq_t[64:128, 64:128], qsum[p][64:128, :], 1.0 / 64,
                    bias_t[64:128, p:p + 1], ALU.mult, ALU.add,
                )
                blkq.append(bq_t)
                bq_b = pers.tile([P, P], bf16, tag=f"blkqb{p}", name=f"blkqb{p}")
                nc.vector.tensor_copy(bq_b[:], bq_t[:])
                blkqb.append(bq_b)

            gd = [nsp.tile([P, P], f32r, tag=f"gd{p}", name=f"gd{p}") for p in range(NPAIR)]
            ns_state = {}
            for g in range(2):
                kt = nsp.tile([P, 256], f32r, tag=f"kt{g}")
                v0 = nsp.tile([P, 256], f32r, tag=f"v{g}", name=f"v0_{g}")
                vt0 = nsp.tile([P, 256], f32r, tag=f"vt0_{g}", name=f"vt0_{g}")
                ns_state[g] = [kt, v0, vt0]
            for p in range(NPAIR):
                g, h = divmod(p, 2)
                hsl = slice(h * P, (h + 1) * P)
                psG = cps.tile([P, P], f32, tag="psG", name=f"psG{p}")
                nc.tensor.matmul(psG[:], blkq[p][:], blkq[p][:], start=True, stop=True)
                gm = wk.tile([P, P], f32, tag="g", name=f"g{p}")
                nc.scalar.activation(gm[:], psG[:], ACTF.Exp, scale=0.125)
                nc.vector.memset(gm[0:64, 64:128], 0.0)
                nc.vector.memset(gm[64:128, 0:64], 0.0)
                gs = wk.tile([P, 1], f32, tag="gs", name=f"gs{p}")
                nc.vector.reduce_sum(gs[:], gm[:], axis=AX.X)
                gri = wk.tile([P, 1], f32, tag="gri", name=f"gri{p}")
                nc.vector.reciprocal(gri[:], gs[:])
                nc.vector.tensor_scalar_mul(gd[p][:], gm[:], gri[:])
                # per-head scale: rowsum(GD)=1 -> s = 1/max(colsum(GD)) per head
                psc = cps.tile([P, P], f32, tag="psc", name=f"psc{p}")
                nc.tensor.matmul(
                    psc[0:1, :], ones_col[:], gd[p][:], start=True, stop=True)
                cm = wk.tile([1, 2], f32, tag="cm", name=f"cm{p}")
                nc.vector.reduce_max(
                    cm[:], psc[0:1, :].rearrange("p (h l) -> p h l", l=64), axis=AX.X)
                cmi = wk.tile([1, 2], f32, tag="cmi", name=f"cmi{p}")
                nc.vector.reciprocal(cmi[:], cm[:])
                cmib = wk.tile([1, 2], f32r, tag="cmib", name=f"cmib{p}")
                nc.vector.tensor_copy(cmib[:], cmi[:])
                psb = cps.tile([P, 2], f32, tag="psb", name=f"psb{p}")
                nc.tensor.matmul(psb[:], ones_row[:], cmib[:], start=True, stop=True)
                sv = wk.tile([P, 2], f32, tag="sv", name=f"sv{p}")
                nc.vector.tensor_copy(sv[:], psb[:])
                pskt = cps.tile([P, P], f32r, tag="pskt", name=f"pskt{p}")
                nc.tensor.matmul(pskt[:], gd[p][:], identb[:], is_transpose=True)
                kt, v0, vt0 = ns_state[g]
                nc.vector.tensor_copy(kt[:, hsl], pskt[:])
                # v0 = s * K^T, vt0 = s * K  (s per head = per row-half)
                nc.vector.tensor_scalar_mul(
                    v0[0:64, hsl], pskt[0:64, :], sv[0:64, 0:1])
                nc.vector.tensor_scalar_mul(
                    v0[64:128, hsl], pskt[64:128, :], sv[64:128, 1:2])
                nc.vector.tensor_scalar_mul(
                    vt0[0:64, hsl], gd[p][0:64, :], sv[0:64, 0:1])
                nc.vector.tensor_scalar_mul(
                    vt0[64:128, hsl], gd[p][64:128, :], sv[64:128, 1:2])

            cctx.__exit__(None, None, None)
            if debug:
                dstage = pers.tile([P, 2048], f32, tag="dstage")
                for p in range(NPAIR):
                    nc.vector.tensor_copy(dstage[:, p * P:(p + 1) * P], blkq[p][:])
                    nc.vector.tensor_copy(dstage[:, 512 + p * P:512 + (p + 1) * P], gd[p][:])
                for g in range(2):
                    nc.vector.tensor_copy(dstage[:, 1024 + g * 256:1024 + (g + 1) * 256], ns_state[g][1][:])
                    nc.vector.tensor_copy(dstage[:, 1536 + g * 256:1536 + (g + 1) * 256], ns_state[g][2][:])
                nc.sync.dma_start(dbg[:, 0:2048], dstage[:])
            mbank = mps.tile([P, NPAIR, P], f32, tag="mbank")

            # ---- D: interleaved NS iterations + ST + token loop ----
            dctx = tc.tile_pool(name="nsps", bufs=3, space="PSUM")
            nsps = dctx.__enter__()
            dctx2 = tc.tile_pool(name="stps", bufs=2, space="PSUM")
            stps = dctx2.__enter__()
            dctx3 = tc.tile_pool(name="tp2", bufs=2, space="PSUM")
            tp2 = dctx3.__enter__()
            def ns_step(it, g):
                kt, v, vt = ns_state[g]
                halves = [slice(0, P), slice(P, 256)]
                pskv = nsps.tile([P, 256], f32, tag="nsb", name=f"pskv{g}_{it}")
                for h, hs in enumerate(halves):
                    nc.tensor.matmul(pskv[:, hs], kt[:, hs], v[:, hs],
                                     start=(h == 0), stop=(h == 1),
                                     skip_group_check=True)
                pskvt = nsps.tile([P, 256], f32, tag="nsb", name=f"pskvt{g}_{it}")
                for h, hs in enumerate(halves):
                    nc.tensor.matmul(pskvt[:, hs], v[:, hs], kt[:, hs],
                                     start=(h == 0), stop=(h == 1),
                                     skip_group_check=True)
                a1 = nsp.tile([P, 256], f32r, tag=f"a1{g}", name=f"a1{g}_{it}")
                nc.vector.tensor_tensor(a1[:], i7[:], pskv[:], ALU.subtract)
                kvt = nsp.tile([P, 256], f32r, tag=f"kvt{g}", name=f"kvt{g}_{it}")
                nc.scalar.copy(kvt[:], pskvt[:])
                psa2 = nsps.tile([P, 256], f32, tag="nsb", name=f"psa2{g}_{it}")
                for h, hs in enumerate(halves):
                    nc.tensor.matmul(psa2[:, hs], kvt[:, hs], a1[:, hs],
                                     start=(h == 0), stop=(h == 1),
                                     skip_group_check=True)
                a3 = nsp.tile([P, 256], f32r, tag=f"a3{g}", name=f"a3{g}_{it}")
                nc.vector.tensor_tensor(a3[:], i15[:], psa2[:], ALU.subtract)
                psa4 = nsps.tile([P, 256], f32, tag="nsb", name=f"psa4{g}_{it}")
                for h, hs in enumerate(halves):
                    nc.tensor.matmul(psa4[:, hs], kvt[:, hs], a3[:, hs],
                                     start=(h == 0), stop=(h == 1),
                                     skip_group_check=True)
                a5 = nsp.tile([P, 256], f32r, tag=f"a5{g}", name=f"a5{g}_{it}")
                nc.vector.tensor_tensor(a5[:], i13[:], psa4[:], ALU.subtract)
                if it < NS_ITERS - 1:
                    psv = nsps.tile([P, 256], f32, tag="nsb", name=f"psv{g}_{it}")
                    for h, hs in enumerate(halves):
                        nc.tensor.matmul(psv[:, hs], vt[:, hs], a5[:, hs],
                                         start=(h == 0), stop=(h == 1),
                                         skip_group_check=True)
                    vn = nsp.tile([P, 256], f32r, tag=f"v{g}", name=f"vn{g}_{it}")
                    nc.vector.tensor_scalar_mul(vn[:], psv[:], 0.25)
                else:
                    vn = v
                psvt2 = nsps.tile([P, 256], f32, tag="nsb", name=f"psvt2{g}_{it}")
                for h, hs in enumerate(halves):
                    nc.tensor.matmul(psvt2[:, hs], a5[:, hs], vt[:, hs],
                                     start=(h == 0), stop=(h == 1),
                                     skip_group_check=True)
                vtn = nsp.tile([P, 256], f32r, tag=f"vt0_{g}", name=f"vtn{g}_{it}")
                nc.vector.tensor_scalar_mul(vtn[:], psvt2[:], 0.25)
                ns_state[g] = [kt, vn, vtn]

            ns_steps = [(it, g) for it in range(NS_ITERS) for g in range(2)]
            ns_per_j = [2, 2, 2, 2, 1, 1, 1, 1]
            ns_i = 0

            for j in range(NCHP):
                for _ in range(ns_per_j[j]):
                    ns_step(*ns_steps[ns_i])
                    ns_i += 1
                jsl = slice(j * XCH, (j + 1) * XCH)
                for p in range(NPAIR):
                    psT = stps.tile([P, 512], f32, tag="psT", name=f"psT{j}_{p}")
                    nc.tensor.matmul(
                        psT[:], blkqb[p][:], qkvt[:, p, jsl], start=True, stop=True)
                    nc.scalar.activation(
                        st[:, p, jsl], psT[:], ACTF.Exp, scale=0.125,
                        accum_out=cp[p][:, j:j + 1],
                    )
                for sc in range(4):
                    t0 = j * 4 + sc
                    tsl = slice(t0 * P, (t0 + 1) * P)
                    psS = tp2.tile([P, 512], bf16, tag="psS", name=f"psS{t0}")
                    for p in range(NPAIR):
                        nc.tensor.matmul(
                            psS[:, p * P:(p + 1) * P], st[:, p, tsl], identb[:],
                            is_transpose=True, start=(p == 0), stop=(p == NPAIR - 1),
                            skip_group_check=True,
                        )
                    sn = snp.tile([P, 512], bf16, tag="sn", name=f"sn{t0}")
                    nc.vector.tensor_copy(sn[:], psS[:])
                    for p in range(NPAIR):
                        nc.tensor.matmul(
                            mbank[:, p, :], sn[:, p * P:(p + 1) * P],
                            qn[:, t0, p * P:(p + 1) * P],
                            start=(t0 == 0), stop=(t0 == TCH - 1),
                            skip_group_check=True,
                        )

            dctx3.__exit__(None, None, None)
            dctx2.__exit__(None, None, None)
            dctx.__exit__(None, None, None)

            # ---- E: W = V6 @ (diag(1/c) M), plus ones cols for r ----
            ectx = tc.tile_pool(name="wps", bufs=1, space="PSUM")
            wps = ectx.__enter__()
            wpads = []
            for p in range(NPAIR):
                g, h = divmod(p, 2)
                hsl = slice(h * P, (h + 1) * P)
                cs = wk.tile([P, 1], f32, tag="cs", name=f"cs{p}")
                nc.vector.reduce_sum(cs[:], cp[p][:], axis=AX.X)
                cinv = wk.tile([P, 1], f32, tag="cinv", name=f"cinv{p}")
                nc.vector.reciprocal(cinv[:], cs[:])
                dvp = wk.tile([P, P], f32r, tag="dvp", name=f"dvp{p}")
                nc.vector.tensor_scalar_mul(dvp[:], mbank[:, p, :], cinv[:])
                # zero cross-head blocks (S/V are dense across the pair)
                nc.vector.memset(dvp[0:64, 64:128], 0.0)
                nc.vector.memset(dvp[64:128, 0:64], 0.0)
                psw = wps.tile([P, P], f32, tag="psw", name=f"psw{p}")
                _, _, vt6 = ns_state[g]
                nc.tensor.matmul(psw[:], vt6[:, hsl], dvp[:], start=True, stop=True)
                wpad = pers.tile([P, 132], bf16, tag=f"wpad{p}")
                nc.vector.memset(wpad[:], 0.0)
                nc.scalar.copy(wpad[:, 0:P], psw[:])
                nc.vector.memset(wpad[0:64, 128:129], 1.0)
                nc.vector.memset(wpad[64:128, 129:130], 1.0)
                wpads.append(wpad)

            if debug:
                dstage2 = pers.tile([P, 2048], f32, tag="dstage2")
                for g in range(2):
                    nc.vector.tensor_copy(dstage2[:, g * 256:(g + 1) * 256], ns_state[g][2][:])
                for p in range(NPAIR):
                    nc.vector.tensor_copy(dstage2[:, 512 + p * P:512 + (p + 1) * P], mbank[:, p, :])
                    nc.vector.tensor_copy(dstage2[:, 1024 + p * 132:1024 + (p + 1) * 132], wpads[p][:])
                    nc.vector.tensor_copy(dstage2[:, 1600 + p * 8:1600 + (p + 1) * 8], cp[p][:])
                nc.sync.dma_start(dbg[:, 2048:4096], dstage2[:])
                dstage3 = pers.tile([P, 2048], f32, tag="dstage3")
                for p in range(NPAIR):
                    nc.vector.tensor_copy(dstage3[:, p * 128:(p + 1) * 128], qkvt[:, p, 0:128])
                    nc.vector.tensor_copy(dstage3[:, 512 + p * 128:512 + (p + 1) * 128], st[:, p, 0:128])
                nc.vector.tensor_copy(dstage3[:, 1024:1536], qn[:, 0, :])
                nc.sync.dma_start(dbg[:, 4096:6144], dstage3[:])
            # ---- F: out = diag(1/r) S W ----
            fctx = tc.tile_pool(name="fps", bufs=2, space="PSUM")
            fps = fctx.__enter__()
            for t in range(TCH):
                tsl = slice(t * P, (t + 1) * P)
                ot = otp.tile([P, 512], f32, tag="ot", name=f"ot{t}")
                for q in range(2):
                    pso = fps.tile([P, 260], f32, tag="pso", name=f"pso{q}_{t}")
                    for jj in range(2):
                        p = 2 * q + jj
                        nc.tensor.matmul(
                            pso[:, jj * 130:(jj + 1) * 130], st[:, p, tsl],
                            wpads[p][:, 0:130],
                            start=(jj == 0), stop=(jj == 1), skip_group_check=True,
                        )
                    rv4 = wk.tile([P, 2, 2, 1], f32, tag=f"rv{q}", name=f"rv{q}_{t}")
                    data = pso[:].rearrange("p (b x) -> p b x", b=2)
                    if q == 0:
                        nc.vector.reciprocal(
                            rv4[:, :, :, 0], data[:, :, 128:130])
                        nc.vector.tensor_tensor(
                            ot[:, 0:256].rearrange("p (b h d) -> p b h d", h=2, d=64),
                            data[:, :, 0:128].rearrange("p b (h d) -> p b h d", d=64),
                            rv4[:].to_broadcast([P, 2, 2, 64]),
                            ALU.mult,
                        )
                    else:
                        pf = snp.tile([P, 260], f32, tag="pf", name=f"pf{t}")
                        nc.scalar.copy(pf[:], pso[:])
                        dataf = pf[:].rearrange("p (b x) -> p b x", b=2)
                        nc.vector.reciprocal(
                            rv4[:, :, :, 0], dataf[:, :, 128:130])
                        nc.gpsimd.tensor_tensor(
                            ot[:, 256:512].rearrange("p (b h d) -> p b h d", h=2, d=64),
                            dataf[:, :, 0:128].rearrange("p b (h d) -> p b h d", d=64),
                            rv4[:].to_broadcast([P, 2, 2, 64]),
                            ALU.mult,
                        )
                nc.sync.dma_start(out_d[tsl, :], ot[:])
            fctx.__exit__(None, None, None)
            ectx.__exit__(None, None, None)

    nc.compile()
    return nc


def _get_nc():
    if "nc" not in _CACHE:
        _CACHE["nc"] = _build()
    return _CACHE["nc"]


def kernel(X, Wq, bq):
    from concourse.bass_utils import run_bass_kernel_spmd

    nc = _get_nc()
    B, E, n = X.shape
    H = Wq.shape[0]
    in_maps = []
    for core in range(8):
        b = core // 2
        h0 = 8 * (core % 2)
        wq_c = Wq[h0:h0 + 8]                      # [8, 64, 1024]
        wqt_c = np.ascontiguousarray(wq_c.transpose(2, 0, 1).reshape(E, 512))
        bias_c = np.ascontiguousarray(bq[h0:h0 + 8].reshape(512))
        in_maps.append({
            "X": np.ascontiguousarray(X[b]),
            "WqT": wqt_c,
            "bias": bias_c,
        })
    res = run_bass_kernel_spmd(nc, in_maps, core_ids=list(range(8)))
    out = np.empty((B, H, n, 64), dtype=np.float32)
    for core in range(8):
        b = core // 2
        h0 = 8 * (core % 2)
        oc = res.results[core]["out"].reshape(n, 8, 64)
        out[b, h0:h0 + 8] = oc.transpose(1, 0, 2)
    return out
